# revision 1
# baseline (speedup 1.0000x reference)
"""Trainium2 Bass kernel for nn_BaseEncoder (ragged entity-pair encoder).

Contract: kernel(**inputs) takes the FULL unsharded inputs (numpy) and
returns the FULL output [B, Q, E, E, R] float32.

Sharding: B*Q = 8 independent (batch, query) pairs -> one per NeuronCore.
Small weights (W_head / W_tail / prototypes-for-that-b) are replicated.

Host-side prep per core (cheap, index/layout only):
  - gather the E*M mention rows of the per-query attention and sum over the
    M=2 mentions (the /2 and /NH scalings cancel in the later row-softmax-
    style normalization, so they are dropped),
  - transpose to At[l, (h,e)] so the device never needs a transpose,
  - entity means ent = mean_m seq[pos] (transposed to entT),
  - prototypes for this b, reshaped/transposed to [2H, R*P].

Device kernel per core (all fp32):
  mul[l, e*32+f] = sum_h At[l,h,e] * At[l,h,f]              (VectorE)
  S[ef]   = sum_l mul[l, ef]                                 (TensorE, ones)
  ctxT[h', ef] = sum_l seq[l, h'] * mul[l, ef]               (TensorE)
  ctxnT = ctxT * (1/S)                                       (VectorE)
  epH[h'', e] = sum_h' W_head[h', h''] entT[h', e]  (and tail)    (TensorE)
  hT[h'', ef] = tanh(sum_h' W_head[768+h', h''] ctxnT[h', ef] + epH[h'', e])
  tT[h'', ef] = tanh(... W_tail ... + epT[h'', f])       (TensorE+VectorE+ScalarE)
  scores[ef, rp] = sum_d candT[d, ef] * protoT[d, rp]        (TensorE)
  out[ef, r] = max_p scores[ef, r*10+p]                      (VectorE)
"""

import numpy as np

B, Q, L, H, E, M, R, P, NH = 2, 4, 1024, 768, 32, 2, 5, 10, 12
NCORES = 8
LT = L // 128          # 8 l-tiles
HT = H // 128          # 6 tiles of 128 along a hidden dim
EF = E * E             # 1024 entity pairs
RP = R * P             # 50 prototype rows

_CACHE = {}


def _build_program():
    import concourse.mybir as mybir
    import concourse.tile as tile
    from concourse import bacc

    f32 = mybir.dt.float32
    nc = bacc.Bacc("TRN2", target_bir_lowering=False, debug=False,
                   num_devices=NCORES)

    at_d = nc.dram_tensor("at", [L, NH * E], f32, kind="ExternalInput").ap()
    seq_d = nc.dram_tensor("seq", [L, H], mybir.dt.float32r, kind="ExternalInput").ap()
    entT_d = nc.dram_tensor("entT", [H, E], f32, kind="ExternalInput").ap()
    wh_d = nc.dram_tensor("wh", [2 * H, H], mybir.dt.float32r, kind="ExternalInput").ap()
    wt_d = nc.dram_tensor("wt", [2 * H, H], mybir.dt.float32r, kind="ExternalInput").ap()
    ptT_d = nc.dram_tensor("ptT", [2 * H, RP], mybir.dt.float32r, kind="ExternalInput").ap()
    out_d = nc.dram_tensor("out", [EF, R], f32, kind="ExternalOutput").ap()

    with tile.TileContext(nc) as tc:
        _emit(tc, mybir, at_d, seq_d, entT_d, wh_d, wt_d, ptT_d, out_d)

    nc.compile()
    return nc


USE_F32R = True
HC = EF // 2            # 512-wide ef chunk (= one PSUM bank of fp32)


def _emit(tc, mybir, at_d, seq_d, entT_d, wh_d, wt_d, ptT_d, out_d):
    nc = tc.nc
    f32 = mybir.dt.float32
    f32r = mybir.dt.float32r

    Alu = mybir.AluOpType
    Act = mybir.ActivationFunctionType
    Ax = mybir.AxisListType
    from concourse.masks import make_identity

    import contextlib
    ctx = contextlib.ExitStack()
    with ctx:
        const = ctx.enter_context(tc.tile_pool(name="const", bufs=1))
        big = ctx.enter_context(tc.tile_pool(name="big", bufs=1))
        mulp = ctx.enter_context(tc.tile_pool(name="mulp", bufs=12))
        candp = ctx.enter_context(tc.tile_pool(name="candp", bufs=14))
        ctxp = ctx.enter_context(tc.tile_pool(name="ctxp", bufs=2))
        tmp = ctx.enter_context(tc.tile_pool(name="tmp", bufs=3))
        # PSUM: 8 banks statically split into tags
        #   "ctx": 6 x 1 bank   (per-chunk ctx accumulators; later proj-B)
        #   "sg":  1 x 1 bank   (S-gram, recS broadcast, even proj-A groups)
        #   "tail": 1 x 1 bank  (ep, odd proj-A groups, scores, transposes)
        psum = ctx.enter_context(tc.tile_pool(name="psum", bufs=1, space="PSUM"))

        # ---------------- input loads ----------------
        at_sb = big.tile([128, LT, NH * E], f32, tag="at_sb")
        at_r = at_d.rearrange("(t p) n -> p t n", p=128)
        for lt in range(LT):
            nc.sync.dma_start(out=at_sb[:, lt, :], in_=at_r[:, lt, :])
        seq_sb = big.tile([128, LT, H], f32r, tag="seq_sb")
        nc.sync.dma_start(out=seq_sb, in_=seq_d.rearrange("(t p) n -> p t n", p=128))
        entT_sb = const.tile([128, HT, E], f32, tag="entT_sb")
        nc.sync.dma_start(out=entT_sb, in_=entT_d.rearrange("(t p) n -> p t n", p=128))
        ptT_sb = const.tile([128, 2 * HT, RP], f32r, tag="ptT_sb")
        nc.sync.dma_start(out=ptT_sb, in_=ptT_d.rearrange("(t p) n -> p t n", p=128))
        wh_sb = big.tile([128, 2 * HT, H], f32r, tag="wh_sb")
        nc.sync.dma_start(out=wh_sb, in_=wh_d.rearrange("(t p) n -> p t n", p=128))
        wt_sb = big.tile([128, 2 * HT, H], f32r, tag="wt_sb")
        nc.sync.dma_start(out=wt_sb, in_=wt_d.rearrange("(t p) n -> p t n", p=128))

        ones_row = const.tile([1, 128], f32, tag="ones_row")
        nc.vector.memset(ones_row, 1.0)
        ident = const.tile([RP, RP], f32, tag="ident")
        make_identity(nc, ident)
        recd = nc.dram_tensor("recd", [E, E], f32).ap()

        # ------- S via Gram over the raw At slices (independent of mul) ----
        # S[e, f] = sum_{h, l} At[l, (h, e)] * At[l, (h, f)]
        sg_ps = psum.tile([E, E], f32, tag="sg", bufs=1, name="sg_ps")
        n_acc = LT * NH
        k = 0
        for lt in range(LT):
            for h in range(NH):
                sl = at_sb[:, lt, h * E:(h + 1) * E]
                nc.tensor.matmul(sg_ps, sl, sl, start=(k == 0),
                                 stop=(k == n_acc - 1))
                k += 1
        r2_sb = const.tile([E, E], f32, tag="r2_sb")
        nc.scalar.copy(r2_sb, sg_ps)
        nc.vector.reciprocal(r2_sb, r2_sb)
        # flatten [32, 32] -> [1, 1024] via a DRAM bounce, then broadcast to
        # all 128 partitions with ones[1,128].T @ chunk.
        nc.sync.dma_start(out=recd, in_=r2_sb)
        rec1 = const.tile([1, EF], f32, tag="rec1")
        nc.sync.dma_start(out=rec1,
                          in_=recd.rearrange("a b -> (a b)")[None, :])
        recS_sb = big.tile([128, EF], f32, tag="recS_sb")
        for c in range(2):
            rb = psum.tile([128, HC], f32, tag="sg", bufs=1, name="recB")
            nc.tensor.matmul(rb, ones_row, rec1[:, c * HC:(c + 1) * HC],
                             start=True, stop=True)
            nc.scalar.copy(recS_sb[:, c * HC:(c + 1) * HC], rb)

        # ---------------- entity projections (ent @ W[:H]) ----------------
        ep_sb = []
        for w, wsb in ((0, wh_sb), (1, wt_sb)):
            ep = const.tile([128, HT, E], f32, tag=f"ep{w}", name=f"ep{w}")
            ep_sb.append(ep)
            for ht2 in range(HT):
                ps = psum.tile([128, E], f32, tag="tail", bufs=1, name="ep_ps")
                for kt in range(HT):
                    nc.tensor.matmul(
                        ps, wsb[:, kt, ht2 * 128:(ht2 + 1) * 128].bitcast(f32),
                        entT_sb[:, kt, :],
                        start=(kt == 0), stop=(kt == HT - 1))
                nc.scalar.copy(ep[:, ht2, :], ps)

        # ---------------- chunked main pipeline ----------------
        # Chunk c covers pairs ef in [c*512, (c+1)*512) i.e. e in [16c, 16c+16).
        EC = E // 2

        def emit_mul_chunk(c, lt, mulA=None):
            """VectorE: mul products+adds for chunk c, l-tile lt.

            Chunk 1 exploits symmetry: its f<16 half equals the transpose of
            chunk 0's f>=16 half, so only the (e>=16, f>=16) quadrant is
            computed; the rest is one strided copy from the chunk-0 tile.
            """
            at3 = at_sb[:, lt, :].rearrange("p (h e) -> p h e", h=NH)
            mt = mulp.tile([128, HC], f32r, tag="mul", name=f"mul{c}_{lt}")
            m3 = mt.rearrange("p (e f) -> p e f", e=EC)
            es = c * EC
            fs = 0 if c == 0 else EC
            FW = E - fs
            for h in range(NH):
                a_e = at3[:, h, es:es + EC, None].broadcast_to([128, EC, FW])
                a_f = at3[:, h, None, fs:].broadcast_to([128, EC, FW])
                if h == 0:
                    nc.vector.tensor_mul(m3[:, :, fs:], a_e, a_f)
                else:
                    t = tmp.tile([128, EC, E], f32, tag="scratch",
                                 name="prod")
                    tq = t[:, :, :FW]
                    nc.vector.tensor_mul(tq, a_e, a_f)
                    nc.vector.tensor_add(m3[:, :, fs:], m3[:, :, fs:], tq)
            if c == 1:
                # m3[e2, f1] = mulA[f1, 16+e2] for f1 < 16 (Gram symmetry)
                w = mulA.rearrange("p (e f) -> p e f", e=EC)[:, :, EC:]
                nc.vector.tensor_copy(m3[:, :, :EC],
                                      w.rearrange("p a b -> p b a"))
            return mt

        def emit_ctx_chunk(c, lt, mt, ctx_ps):
            for ht in range(HT):
                nc.tensor.matmul(
                    ctx_ps[ht], seq_sb[:, lt, ht * 128:(ht + 1) * 128],
                    mt, start=(lt == 0), stop=(lt == LT - 1))

        def emit_norm_chunk(c, ctx_ps):
            cn = ctxp.tile([128, HT, HC], f32r, tag="ctxn", name=f"ctxn{c}")
            for ht in range(HT):
                nc.vector.tensor_mul(cn[:, ht, :], ctx_ps[ht],
                                     recS_sb[:, c * HC:(c + 1) * HC])
            return cn

        def emit_proj_group(c, g, cn, cand_t, ps_tag):
            w, ht2 = divmod(g, HT)
            wsb = wh_sb if w == 0 else wt_sb
            nb = HT if ps_tag == "ctx" else 1
            ps = psum.tile([128, HC], f32, tag=ps_tag, bufs=nb,
                           name=f"proj{c}_{g}")
            for kt in range(HT):
                nc.tensor.matmul(ps, wsb[:, HT + kt, ht2 * 128:(ht2 + 1) * 128],
                                 cn[:, kt, :],
                                 start=(kt == 0), stop=(kt == HT - 1))
            es = c * EC
            if w == 0:
                bias = ep_sb[0][:, ht2, es:es + EC, None].broadcast_to(
                    [128, EC, E])
            else:
                bias = ep_sb[1][:, ht2, None, :].broadcast_to([128, EC, E])
            pre = tmp.tile([128, EC, E], f32, tag="scratch", name="pre")
            nc.vector.tensor_add(pre, ps.rearrange("p (e f) -> p e f", e=EC),
                                 bias)
            cd = candp.tile([128, HC], f32r, tag="cand", name=f"cand{c}_{g}")
            cand_t[g] = cd
            nc.scalar.activation(cd, pre.rearrange("p a b -> p (a b)"),
                                 Act.Tanh)

        def emit_scores_chunk(c, cand_t, ps_tag):
            sc = psum.tile([RP, HC], f32, tag=ps_tag, bufs=1, name=f"sc{c}")
            order = [w * HT + kt for w in range(2) for kt in range(HT)]
            for i, g in enumerate(order):
                nc.tensor.matmul(sc, ptT_sb[:, g, :], cand_t[g],
                                 start=(i == 0), stop=(i == 2 * HT - 1))
            scT = const.tile([RP, HC], f32, tag=f"scT{c}", name=f"scT{c}")
            nc.scalar.copy(scT, sc)
            ob = const.tile([128, LT // 2, R], f32, tag=f"ob{c}",
                            name=f"ob{c}")
            for et in range(LT // 2):
                tp = psum.tile([128, RP], f32, tag="sg", bufs=1, name="tp")
                nc.tensor.transpose(tp, scT[:, et * 128:(et + 1) * 128],
                                    ident)
                nc.vector.tensor_reduce(
                    out=ob[:, et, :],
                    in_=tp.rearrange("p (r q) -> p r q", r=R),
                    axis=Ax.X, op=Alu.max)
            nc.sync.dma_start(
                out=out_d.rearrange("(t p) r -> p t r", p=128)[
                    :, c * (LT // 2):(c + 1) * (LT // 2), :],
                in_=ob)

        # ---- phase A: mul+ctx for chunk 0 ----
        ctxA_ps = [psum.tile([128, HC], f32, tag="ctx", bufs=HT,
                             name=f"ctxA{ht}") for ht in range(HT)]
        mulA_t = []
        for lt in range(LT):
            mt = emit_mul_chunk(0, lt)
            mulA_t.append(mt)
            emit_ctx_chunk(0, lt, mt, ctxA_ps)
        cnA = emit_norm_chunk(0, ctxA_ps)

        # ---- phase B: mul+ctx for chunk 1, interleaved with chunk-0 tail ---
        candA = [None] * (2 * HT)
        ctxB_ps = [psum.tile([128, HC], f32, tag="ctx", bufs=HT,
                             name=f"ctxB{ht}") for ht in range(HT)]
        projA_sched = {1: [0, 1], 2: [2, 3], 3: [4, 5], 4: [6, 7],
                       5: [8, 9], 6: [10, 11]}
        for lt in range(LT):
            mt = emit_mul_chunk(1, lt, mulA=mulA_t[lt])
            emit_ctx_chunk(1, lt, mt, ctxB_ps)
            for g in projA_sched.get(lt, []):
                emit_proj_group(0, g, cnA, candA, "sg" if g % 2 == 0
                                else "tail")
        emit_scores_chunk(0, candA, "tail")
        cnB = emit_norm_chunk(1, ctxB_ps)

        # ---- chunk-1 tail (PE slots from the freed ctx accumulators) ----
        candB = [None] * (2 * HT)
        for g in range(2 * HT):
            emit_proj_group(1, g, cnB, candB, "ctx")
        emit_scores_chunk(1, candB, "tail")


def _host_prep(sequence_output, attention, W_head, W_tail, prototypes,
               mention_pos):
    """Build the per-core input maps (numpy only)."""
    seq = np.ascontiguousarray(sequence_output, dtype=np.float32)
    att = np.asarray(attention, dtype=np.float32)
    wh = np.ascontiguousarray(W_head, dtype=np.float32)
    wt = np.ascontiguousarray(W_tail, dtype=np.float32)
    pro = np.asarray(prototypes, dtype=np.float32)
    pos = np.asarray(mention_pos)

    in_maps = []
    for c in range(NCORES):
        b, q = divmod(c, Q)
        p_bq = pos[b, q]                       # [E, M]
        # attention gather + mention-sum: [NH, E, L] (scale dropped)
        g = att[b, q][:, p_bq, :]              # [NH, E, M, L]
        asum = g[:, :, 0, :] + g[:, :, 1, :]   # [NH, E, L]
        at = np.ascontiguousarray(
            asum.reshape(NH * E, L).T)         # [L, NH*E], At[l, h*E+e]
        # entity means: [E, H] -> entT [H, E]
        ment = seq[b, q][p_bq]                 # [E, M, H]
        ent = (ment[:, 0, :] + ment[:, 1, :]) * np.float32(0.5)
        entT = np.ascontiguousarray(ent.T)
        ptT = np.ascontiguousarray(
            pro[b].reshape(RP, 2 * H).T)       # [2H, RP]
        in_maps.append({
            "at": at,
            "seq": seq[b, q],
            "entT": entT,
            "wh": wh,
            "wt": wt,
            "ptT": ptT,
        })
    return in_maps


def kernel(sequence_output, attention, W_head, W_tail, prototypes,
           mention_pos):
    from concourse.bass_utils import run_bass_kernel_spmd

    if "nc" not in _CACHE:
        _CACHE["nc"] = _build_program()
    nc = _CACHE["nc"]

    in_maps = _host_prep(sequence_output, attention, W_head, W_tail,
                         prototypes, mention_pos)
    res = run_bass_kernel_spmd(nc, in_maps, core_ids=list(range(NCORES)))

    out = np.empty((B, Q, E, E, R), dtype=np.float32)
    for c in range(NCORES):
        b, q = divmod(c, Q)
        out[b, q] = res.results[c]["out"].reshape(E, E, R)
    return out



# revision 18
# speedup vs baseline: 1.6641x; 1.6641x over previous
"""Trainium2 Bass kernel for nn_BaseEncoder (ragged entity-pair encoder).

Contract: kernel(**inputs) takes the FULL unsharded inputs (numpy) and
returns the FULL output [B, Q, E, E, R] float32.

Sharding: B*Q = 8 independent (batch, query) pairs -> one per NeuronCore.
Small weights (W_head / W_tail / prototypes-for-that-b) are replicated.

Host-side prep per core (cheap, index/layout only):
  - gather the E*M mention rows of the per-query attention and sum over the
    M=2 mentions (the /2 and /NH scalings cancel in the later normalization),
  - layout At[l, (e, h)] (h innermost!) in bf16 so the device outer-products
    run with packed access patterns (DVE 2x mode),
  - entity means ent = mean_m seq[pos] (transposed to entT),
  - prototypes for this b, reshaped/transposed to [2H, R*P],
  - tiny constant masks for the PE bias-broadcast matmuls.

Device kernel per core (bf16 compute, fp32 PSUM):
  ZZ[l, e, f, h] = At[l,(e,h)] * At[l,(f,h)]          (VectorE, 2x packed)
  mul[l, (e,f)] = tree-sum_h ZZ                        (VectorE, mostly 2x)
  S[e,f]   = Gram over (l,h) of At                     (TensorE)
  ctx[h', ef] = sum_l seq[l,h'] * mul[l, ef]           (TensorE)
  cn = ctx * (1/S)                                     (ScalarE copy + VectorE)
  epT[e, h''] = ent @ W[:H]                            (TensorE)
  pre[h'', ef] = W[H:].T @ cn + mask-matmul(epT)       (TensorE, bias in PSUM)
  cand = tanh(pre)                                     (ScalarE, direct PSUM)
  scores[ef, rp] = cand.T @ ptT                        (TensorE, [ef,rp] orient)
  out[ef, r] = max_p scores[ef, r*10+p]                (VectorE reduce)
"""

import numpy as np
import ml_dtypes

B, Q, L, H, E, M, R, P, NH = 2, 4, 1024, 768, 32, 2, 5, 10, 12
NCORES = 8
LT = L // 128          # 8 l-tiles
HT = H // 128          # 6 tiles of 128 along a hidden dim
EF = E * E             # 1024 entity pairs
RP = R * P             # 50 prototype rows
HC = EF // 2           # 512-wide ef chunk (= one PSUM bank of fp32)
EC = E // 2            # 16 e-rows per chunk

_CACHE = {}


def _build_program():
    import concourse.mybir as mybir
    import concourse.tile as tile
    from concourse import bacc

    bf16 = mybir.dt.bfloat16
    f32 = mybir.dt.float32
    nc = bacc.Bacc("TRN2", target_bir_lowering=False, debug=False,
                   num_devices=NCORES)

    at_d = nc.dram_tensor("at", [L, E * NH], bf16, kind="ExternalInput").ap()
    seq_d = nc.dram_tensor("seq", [L, H], bf16, kind="ExternalInput").ap()
    entT_d = nc.dram_tensor("entT", [H, E], bf16, kind="ExternalInput").ap()
    wh_d = nc.dram_tensor("wh", [2 * H, H], bf16, kind="ExternalInput").ap()
    wt_d = nc.dram_tensor("wt", [2 * H, H], bf16, kind="ExternalInput").ap()
    ptT_d = nc.dram_tensor("ptT", [2 * H, RP], bf16, kind="ExternalInput").ap()
    m16_d = nc.dram_tensor("m16", [2, E, HC], bf16, kind="ExternalInput").ap()
    m32_d = nc.dram_tensor("m32", [E, HC], bf16, kind="ExternalInput").ap()
    out_d = nc.dram_tensor("out", [EF, R], f32, kind="ExternalOutput").ap()

    with tile.TileContext(nc) as tc:
        _emit(tc, mybir, at_d, seq_d, entT_d, wh_d, wt_d, ptT_d, m16_d,
              m32_d, out_d)

    nc.compile()
    return nc


def _emit(tc, mybir, at_d, seq_d, entT_d, wh_d, wt_d, ptT_d, m16_d, m32_d,
          out_d):
    nc = tc.nc
    bf16 = mybir.dt.bfloat16
    f32 = mybir.dt.float32
    Alu = mybir.AluOpType
    Act = mybir.ActivationFunctionType
    Ax = mybir.AxisListType

    import contextlib
    ctx = contextlib.ExitStack()
    with ctx:
        const = ctx.enter_context(tc.tile_pool(name="const", bufs=1))
        big = ctx.enter_context(tc.tile_pool(name="big", bufs=1))
        zz = ctx.enter_context(tc.tile_pool(name="zz", bufs=2))
        mulp = ctx.enter_context(tc.tile_pool(name="mulp", bufs=12))
        ctxp = ctx.enter_context(tc.tile_pool(name="ctxp", bufs=1))
        candp = ctx.enter_context(tc.tile_pool(name="candp", bufs=14))
        psum = ctx.enter_context(tc.tile_pool(name="psum", bufs=1,
                                              space="PSUM"))

        # ---------------- input loads (SP HWDGE queue) ----------------
        at_sb = big.tile([128, LT, E, NH], bf16, tag="at_sb")
        at_r = at_d.rearrange("(t p) n -> p t n", p=128)
        for lt in range(LT):
            nc.sync.dma_start(
                out=at_sb[:, lt].rearrange("p e h -> p (e h)"),
                in_=at_r[:, lt])
        seq_sb = big.tile([128, LT, H], bf16, tag="seq_sb")
        nc.sync.dma_start(out=seq_sb,
                          in_=seq_d.rearrange("(t p) n -> p t n", p=128))
        entT_sb = const.tile([128, HT, E], bf16, tag="entT_sb")
        nc.sync.dma_start(out=entT_sb,
                          in_=entT_d.rearrange("(t p) n -> p t n", p=128))
        wh_sb = big.tile([128, 2 * HT, H], bf16, tag="wh_sb")
        wt_sb = big.tile([128, 2 * HT, H], bf16, tag="wt_sb")
        wh_r = wh_d.rearrange("(t p) n -> p t n", p=128)
        wt_r = wt_d.rearrange("(t p) n -> p t n", p=128)
        for w_sb, w_r in ((wh_sb, wh_r), (wt_sb, wt_r)):
            nc.sync.dma_start(out=w_sb[:, 0:HT], in_=w_r[:, 0:HT])
        m16_sb = [const.tile([E, HC], bf16, tag=f"m16_sb{c}", name=f"m16_{c}")
                  for c in range(2)]
        for c in range(2):
            nc.sync.dma_start(out=m16_sb[c], in_=m16_d[c])
        m32_sb = const.tile([E, HC], bf16, tag="m32_sb")
        nc.sync.dma_start(out=m32_sb, in_=m32_d)
        for w_sb, w_r in ((wh_sb, wh_r), (wt_sb, wt_r)):
            nc.sync.dma_start(out=w_sb[:, HT:], in_=w_r[:, HT:])
        ptT_sb = const.tile([128, 2 * HT, RP], bf16, tag="ptT_sb")
        nc.sync.dma_start(out=ptT_sb,
                          in_=ptT_d.rearrange("(t p) n -> p t n", p=128))

        ones_row = const.tile([1, 128], bf16, tag="ones_row")
        nc.vector.memset(ones_row, 1.0)
        recd = nc.dram_tensor("recd", [E, E], bf16).ap()

        # ---------------- DVE: products + h-sum tree ----------------
        def emit_mul(lt, c, mulA=None):
            """mul[l, (e,f)] for chunk c, l-tile lt (bf16, 2x-packed ops).

            Chunk 0: e in [0,16), all f.  Chunk 1: e in [16,32): only the
            f>=16 quadrant is computed; f<16 is the transpose of chunk 0's
            right half (Gram symmetry).
            """
            v = at_sb[:, lt]                    # [128, E, NH]
            FW = E if c == 0 else EC
            es = c * EC
            fs = 0 if c == 0 else EC
            z = zz.tile([128, EC, FW, NH], bf16, tag=f"zz{c}",
                        name=f"zz{c}_{lt}")
            nc.vector.tensor_mul(
                z,
                v[:, es:es + EC, None, :].broadcast_to([128, EC, FW, NH]),
                v[:, None, fs:, :].broadcast_to([128, EC, FW, NH]))
            t4 = zz.tile([128, EC, FW, 4], bf16, tag=f"t4{c}", name=f"t4{c}")
            nc.vector.tensor_add(t4, z[:, :, :, 0:4], z[:, :, :, 4:8])
            nc.vector.tensor_add(t4, t4, z[:, :, :, 8:12])
            u2 = zz.tile([128, EC, FW, 2], bf16, tag=f"u2{c}", name=f"u2{c}")
            nc.vector.tensor_add(u2, t4[:, :, :, 0:2], t4[:, :, :, 2:4])
            mt = mulp.tile([128, HC], bf16, tag="mul", name=f"mul{c}_{lt}")
            m3 = mt.rearrange("p (e f) -> p e f", e=EC)
            nc.vector.tensor_add(m3[:, :, fs:], u2[:, :, :, 0],
                                 u2[:, :, :, 1])
            if c == 1:
                w = mulA.rearrange("p (e f) -> p e f", e=EC)[:, :, EC:]
                nc.vector.tensor_copy(m3[:, :, :EC],
                                      w.rearrange("p a b -> p b a"))
            return mt

        # ---------------- phase A: chunk-0 mul + ctx, gram ----------------
        sg_ps = psum.tile([E, E], f32, tag="sg", bufs=1, name="sg_ps")
        ctxA_ps = [psum.tile([128, HC], f32, tag="ctx", bufs=HT,
                             name=f"ctxA{ht}") for ht in range(HT)]
        r2f = const.tile([E, E], f32, tag="r2f")
        r2b = const.tile([E, E], bf16, tag="r2b")
        # Gram first in PE order: it only gates on the at DMAs, so S is ready
        # ~t12 and the reciprocal/broadcast chain hides under phase A.
        k = 0
        for lt in range(LT):
            for h in range(NH):
                sl = at_sb[:, lt, :, h]
                nc.tensor.matmul(sg_ps, sl, sl, start=(k == 0),
                                 stop=(k == LT * NH - 1))
                k += 1
        nc.scalar.copy(r2f, sg_ps)

        rec1 = const.tile([1, EF], bf16, tag="rec1")
        mulA_t = []
        for lt in range(LT):
            mt = emit_mul(lt, 0)
            mulA_t.append(mt)
            if lt == 2:
                # DVE reaches this ~t20; r2f was ready ~t13
                nc.vector.reciprocal(r2f, r2f)
                nc.vector.tensor_copy(r2b, r2f)
                # SP (idle after loads): bounce [32,32] -> [1,1024] row
                nc.sync.dma_start(out=recd, in_=r2b)
                nc.sync.dma_start(
                    out=rec1, in_=recd.rearrange("a b -> (a b)")[None, :])
            for ht in range(HT):
                nc.tensor.matmul(ctxA_ps[ht],
                                 seq_sb[:, lt, ht * 128:(ht + 1) * 128],
                                 mt, start=(lt == 0), stop=(lt == LT - 1))

        # ---------------- epT: ent @ W[:H]  ->  [E, 768] bf16 -------------
        epT_sb = []
        for w, wsb in ((0, wh_sb), (1, wt_sb)):
            ep = const.tile([E, H], bf16, tag=f"epT{w}", name=f"epT{w}")
            epT_sb.append(ep)
            for cs, cw in ((0, 512), (512, 256)):
                ps = psum.tile([E, cw], f32, tag="tail", bufs=1,
                               name=f"epT_ps{w}_{cs}")
                for kt in range(HT):
                    nc.tensor.matmul(ps, entT_sb[:, kt, :],
                                     wsb[:, kt, cs:cs + cw],
                                     start=(kt == 0), stop=(kt == HT - 1))
                nc.scalar.copy(ep[:, cs:cs + cw], ps)

        # recS broadcast to all 128 partitions via ones-matmul
        recS_sb = const.tile([128, EF], bf16, tag="recS_sb")
        for c in range(2):
            rb = psum.tile([128, HC], f32, tag="tail", bufs=1, name="recB")
            nc.tensor.matmul(rb, ones_row, rec1[:, c * HC:(c + 1) * HC],
                             start=True, stop=True)
            nc.scalar.copy(recS_sb[:, c * HC:(c + 1) * HC], rb)

        def emit_norm(c, ctx_ps):
            cn = ctxp.tile([128, HT, HC], bf16, tag=f"cn{c}", name=f"cn{c}")
            for ht in range(HT):
                nc.scalar.copy(cn[:, ht], ctx_ps[ht])
            rs = recS_sb[:, None, c * HC:(c + 1) * HC]
            nc.vector.tensor_mul(cn, cn, rs.broadcast_to([128, HT, HC]))
            return cn

        def emit_proj_group(c, g, cn, cand_t, ps_tag, ps_bufs=1):
            w, ht2 = divmod(g, HT)
            wsb = wh_sb if w == 0 else wt_sb
            ps = psum.tile([128, HC], f32, tag=ps_tag, bufs=ps_bufs,
                           name=f"proj{c}_{g}")
            for kt in range(HT):
                nc.tensor.matmul(ps, wsb[:, HT + kt, ht2 * 128:(ht2 + 1) * 128],
                                 cn[:, kt], start=(kt == 0), stop=False)
            hs = ht2 * 128
            if w == 0:
                nc.tensor.matmul(ps, epT_sb[0][:, hs:hs + 128],
                                 m16_sb[c], start=False, stop=True)
            else:
                nc.tensor.matmul(ps, epT_sb[1][:, hs:hs + 128],
                                 m32_sb, start=False, stop=True)
            cd = candp.tile([128, HC], bf16, tag="cand", name=f"cand{c}_{g}")
            cand_t[g] = cd
            nc.scalar.activation(cd, ps, Act.Tanh)

        def emit_scores_mm(sc_ps, g, cand_t):
            # One PSUM bank holds all 4 efb accumulation regions. start=True
            # zeroes the WHOLE bank, so only the very first matmul may carry
            # it; the other chains accumulate onto the zeroed bank.
            for efb in range(4):
                nc.tensor.matmul(
                    sc_ps[:, efb],
                    cand_t[g][:, efb * 128:(efb + 1) * 128],
                    ptT_sb[:, g, :], start=(g == 0 and efb == 0),
                    stop=(g == 2 * HT - 1))

        def emit_out(c, sc_ps):
            ob = const.tile([128, 4, R], f32, tag=f"ob{c}", name=f"ob{c}")
            nc.vector.tensor_reduce(
                out=ob, in_=sc_ps.rearrange("p b (r q) -> p b r q", r=R),
                axis=Ax.X, op=Alu.max)
            nc.gpsimd.dma_start(
                out=out_d.rearrange("(t p) r -> p t r", p=128)[
                    :, c * 4:(c + 1) * 4, :],
                in_=ob)

        # ---- phase B: chunk-1 mul + ctx, interleaved with chunk-0 tail ---
        ctxB_ps = [psum.tile([128, HC], f32, tag="ctx", bufs=HT,
                             name=f"ctxB{ht}") for ht in range(HT)]
        candA = [None] * (2 * HT)
        scA = None
        cnA = None
        projA_sched = {1: [0, 1], 2: [2, 3], 3: [4, 5], 4: [6, 7],
                       5: [8, 9], 6: [10, 11]}
        for lt in range(LT):
            mt = emit_mul(lt, 1, mulA=mulA_t[lt])
            if lt == 1:
                cnA = emit_norm(0, ctxA_ps)
            for ht in range(HT):
                nc.tensor.matmul(ctxB_ps[ht],
                                 seq_sb[:, lt, ht * 128:(ht + 1) * 128],
                                 mt, start=(lt == 0), stop=(lt == LT - 1))
            for g in projA_sched.get(lt, []):
                emit_proj_group(0, g, cnA, candA,
                                "sg" if g % 2 == 0 else "tail")
                if g == 2 * HT - 1:
                    scA = psum.tile([128, 4, RP], f32, tag="sg", bufs=1,
                                    name="scA")
                    for gg in range(2 * HT):
                        emit_scores_mm(scA, gg, candA)
        cnB = emit_norm(1, ctxB_ps)
        emit_out(0, scA)

        # ---- phase C: chunk-1 tail (PE slots from freed ctx banks) ------
        # scores matmuls trail the proj groups by one so the PE never waits
        # on the ScalarE tanh of the group it just produced.
        candB = [None] * (2 * HT)
        scB = psum.tile([128, 4, RP], f32, tag="tail", bufs=1, name="scB")
        for g in range(2 * HT):
            emit_proj_group(1, g, cnB, candB, "ctx", ps_bufs=HT)
            if g >= 1:
                emit_scores_mm(scB, g - 1, candB)
        emit_scores_mm(scB, 2 * HT - 1, candB)
        emit_out(1, scB)


def _host_prep(sequence_output, attention, W_head, W_tail, prototypes,
               mention_pos):
    """Build the per-core input maps (numpy only)."""
    bf16 = ml_dtypes.bfloat16
    seq = np.asarray(sequence_output, dtype=np.float32)
    att = np.asarray(attention, dtype=np.float32)
    wh = np.asarray(W_head, dtype=np.float32).astype(bf16)
    wt = np.asarray(W_tail, dtype=np.float32).astype(bf16)
    pro = np.asarray(prototypes, dtype=np.float32)
    pos = np.asarray(mention_pos)

    # PE bias-broadcast masks, contracted against the full [E, .] epT rows:
    # m16[c, k, (e',f)] = (k == 16c + e') broadcasts ep_head[:, 16c+e'] over f;
    # m32[j, (e,f)] = (f == j) broadcasts ep_tail[:, f] over e.
    m16 = np.zeros((2, E, HC), dtype=bf16)
    for c in range(2):
        for i in range(EC):
            m16[c, 16 * c + i, i * E:(i + 1) * E] = 1
    m32 = np.ascontiguousarray(
        np.tile(np.eye(E, dtype=bf16), (1, EC)).reshape(E, HC))

    in_maps = []
    for c in range(NCORES):
        b, q = divmod(c, Q)
        p_bq = pos[b, q]                       # [E, M]
        # attention gather + mention-sum: [NH, E, L] (scale dropped)
        g = att[b, q][:, p_bq, :]              # [NH, E, M, L]
        asum = g[:, :, 0, :] + g[:, :, 1, :]   # [NH, E, L]
        # At[l, (e, h)] with h innermost (packed products on device)
        at = np.ascontiguousarray(
            asum.transpose(2, 1, 0).reshape(L, E * NH)).astype(bf16)
        # entity means: [E, H] -> entT [H, E]
        ment = seq[b, q][p_bq]                 # [E, M, H]
        ent = (ment[:, 0, :] + ment[:, 1, :]) * np.float32(0.5)
        entT = np.ascontiguousarray(ent.T).astype(bf16)
        ptT = np.ascontiguousarray(
            pro[b].reshape(RP, 2 * H).T).astype(bf16)
        in_maps.append({
            "at": at,
            "seq": seq[b, q].astype(bf16),
            "entT": entT,
            "wh": wh,
            "wt": wt,
            "ptT": ptT,
            "m16": m16,
            "m32": m32,
        })
    return in_maps


def kernel(sequence_output, attention, W_head, W_tail, prototypes,
           mention_pos):
    from concourse.bass_utils import run_bass_kernel_spmd

    if "nc" not in _CACHE:
        _CACHE["nc"] = _build_program()
    nc = _CACHE["nc"]

    in_maps = _host_prep(sequence_output, attention, W_head, W_tail,
                         prototypes, mention_pos)
    res = run_bass_kernel_spmd(nc, in_maps, core_ids=list(range(NCORES)))

    out = np.empty((B, Q, E, E, R), dtype=np.float32)
    for c in range(NCORES):
        b, q = divmod(c, Q)
        out[b, q] = res.results[c]["out"].reshape(E, E, R)
    return out
